# revision 20
# baseline (speedup 1.0000x reference)
"""AFT-simple attention (nn_AsfAttention) on 8 TRN2 NeuronCores.

Reference (per batch b):
    emb_q = q @ w_q; emb_k = k @ w_k; emb_v = v @ w_v
    k_exp = exp(emb_k)
    y = sigmoid(emb_q) * cumsum(k_exp * emb_v, seq) / cumsum(k_exp, seq)
    out = y @ w_p

Sharding (v2): core c = 2*b + g handles batch b and OUT-CHANNEL half g
(512 of 1024 channels) over the FULL 8192 sequence.  The per-channel
cumsum is then fully core-local: no carry exchange, no collective, no
mid-kernel stall.  The price is that the final projection out = y @ w_p
only has half the contraction rows per core, so each core emits a
partial [S, H] output and the host sums the two partials per batch
(cheap numpy add during unshard).

Precision: q and k projections run fp8-e4m3 with DoubleRow perf mode
(2 contraction rows per PE cell -> ~1.8x matmul throughput); their
error is gated by sigmoid (q) and damped by the cumsum ratio (k),
measured 1.3e-2 rel total vs the 2e-2 budget.  v and p projections
stay bf16 (fp8 there costs ~4e-2).  Weights are pre-scaled by 128 for
fp8 (w ~ 0.01 would be subnormal in e4m3); the 1/128 unscale rides the
exp/sigmoid activation's scale input for free.

Layout: channel-major on chip ([ch partitions, seq free]) so the
projections use weights as the stationary operand, the seq cumsum is a
native tensor_tensor_scan along the free dim, and the output
projection takes y tiles as lhsT yielding natural [seq, ch_out] rows.
Everything streams in one phase: per 512-seq chunk do k/v/q
projections + scans + elementwise + out projection, with the out
projection software-pipelined one chunk behind the projections.
"""

import numpy as np
import ml_dtypes

import concourse.bass as bass
import concourse.tile as tile
from concourse import bacc, mybir
from concourse.bass_utils import run_bass_kernel_spmd

B, S, H = 4, 8192, 1024
NCORES = 8
GH = 512                  # out-channel half per core
SC = 512                  # seq chunk (columns per matmul / scan step)
NCH = S // SC             # 16 seq chunks
MO = GH // 128            # 4 out-channel tiles per core
KI = H // 128             # 8 contraction subtiles (bf16 path)
KI2 = H // 256            # 4 fp8 DoubleRow contraction pairs
NT = SC // 128            # 4 seq subtiles per chunk for the out projection
WSCALE = 128.0            # fp8 weight pre-scale

bf16 = mybir.dt.bfloat16
f8 = mybir.dt.float8e4
f32 = mybir.dt.float32
AF = mybir.ActivationFunctionType
OP = mybir.AluOpType
DR = mybir.MatmulPerfMode.DoubleRow

_cache = {}


def prune_pe_incs(nc, verbose=False):
    """Drop the per-matmul PE semaphore increment from every matmul that
    is not the end of its accumulation group (stop_tensor_calc), then
    renumber all waits on that semaphore. A wait whose original target
    inc was dropped is rounded UP to the next kept inc — safe here
    because kept-inc matmuls never depend on the rounded-up waiters
    (they only consume earlier-generation tiles).

    Each inc is a serialized EVT_SEM register write (~26 ns) on the PE
    sequencer; at ~1500 matmuls this is ~40 us of PE issue overhead.
    """
    import bisect
    from collections import defaultdict

    insts = []
    for bb in nc.main_func.blocks:
        insts.extend(bb.instructions)

    upd = defaultdict(list)
    for pos, ins in enumerate(insts):
        si = ins.sync_info
        if not si:
            continue
        for u in si.on_update:
            upd[u.ant_name].append((pos, ins, u))

    changed = 0
    for sem, us in upd.items():
        if len(us) < 64:
            continue
        if not all(type(i).__name__ == "InstMatmult"
                   and u.update_mode == "sem-inc" and u.update_value == 1
                   for _, i, u in us):
            continue
        kept = [bool(i.stop_tensor_calc) for _, i, _ in us]
        kept[-1] = True
        kept_idx = [j for j, k in enumerate(kept) if k]

        def new_thresh(t):
            j = bisect.bisect_left(kept_idx, t - 1)
            assert j < len(kept_idx), f"wait {t} beyond last kept inc"
            return j + 1

        for ins in insts:
            si = ins.sync_info
            if not si:
                continue
            for w in si.on_wait:
                if w.ant_name == sem:
                    assert w.wait_mode == "sem-ge-imm", w.wait_mode
                    w.wait_value = new_thresh(w.wait_value)
        for j, (_, ins, u) in enumerate(us):
            if not kept[j]:
                si = ins.sync_info
                rest = [x for x in si.on_update if x.ant_name != sem]
                assert len(rest) == len(si.on_update) - 1
                if rest:
                    si.on_update[:] = rest
                else:
                    si.on_update.clear()
                changed += 1
    if verbose:
        print(f"prune_pe_incs: removed {changed} matmul sem-incs")
    return changed


def build(debug=False):
    nc = bacc.Bacc("TRN2", target_bir_lowering=False, debug=debug,
                   num_devices=1)

    qT_e = nc.dram_tensor("qT8", [H, S], f8, kind="ExternalInput")
    kT_e = nc.dram_tensor("kT8", [H, S], f8, kind="ExternalInput")
    vT_e = nc.dram_tensor("vT", [H, S], bf16, kind="ExternalInput")
    wq_e = nc.dram_tensor("wq8", [H, GH], f8, kind="ExternalInput")
    wk_e = nc.dram_tensor("wk8", [H, GH], f8, kind="ExternalInput")
    wv_e = nc.dram_tensor("wvb", [H, GH], bf16, kind="ExternalInput")
    wp_e = nc.dram_tensor("wpb", [GH, H], bf16, kind="ExternalInput")
    out_e = nc.dram_tensor("out", [S, H], f32, kind="ExternalOutput")

    with tile.TileContext(nc) as tc:
        with (
            tc.tile_pool(name="wts", bufs=1) as wts,
            tc.tile_pool(name="inb", bufs=3) as inb,
            tc.tile_pool(name="act", bufs=2) as actp,
            tc.tile_pool(name="scn", bufs=2) as scn,
            tc.tile_pool(name="sgp", bufs=2) as sgp,
            tc.tile_pool(name="yp", bufs=2) as yp,
            tc.tile_pool(name="tmp", bufs=3) as tmp,
            tc.tile_pool(name="osb", bufs=4) as osbp,
            tc.tile_pool(name="ps", bufs=4, space="PSUM") as ps,
            tc.tile_pool(name="pso", bufs=2, space="PSUM") as pso,
        ):
            # ---- weights (resident all kernel) -----------------------
            # Load order = first-use order: wk + first k chunk head the
            # DMA queues so the PE can start within a few us.
            def load_w(ext, name, dtype):
                t = wts.tile([128, KI if ext is not wp_e else MO,
                              GH if ext is not wp_e else H], dtype, tag=name)
                nsub = KI if ext is not wp_e else MO
                for kk in range(nsub):
                    nc.sync.dma_start(
                        t[:, kk, :], ext[kk * 128:(kk + 1) * 128, :])
                return t

            wk_t = load_w(wk_e, "wk", f8)

            def load_in(ext, tagpfx, dtype, s):
                t = inb.tile([128, KI, SC], dtype, tag=tagpfx)
                for kk in range(KI):
                    nc.sync.dma_start(
                        t[:, kk, :],
                        ext[kk * 128:(kk + 1) * 128, bass.ts(s, SC)])
                return t

            kc0 = load_in(kT_e, "ik", f8, 0)
            wv_t = load_w(wv_e, "wv", bf16)
            vc0 = load_in(vT_e, "iv", bf16, 0)
            wq_t = load_w(wq_e, "wq", f8)
            qc0 = load_in(qT_e, "iq", f8, 0)
            wp_t = load_w(wp_e, "wp", bf16)

            sk_prev = [None] * MO
            skv_prev = [None] * MO

            def kproj(kc, m):
                psm = ps.tile([128, SC], f32, tag="ps")
                for kk in range(KI2):
                    nc.tensor.matmul(
                        psm[:],
                        wk_t[:, 2 * kk:2 * kk + 2, m * 128:(m + 1) * 128],
                        kc[:, 2 * kk:2 * kk + 2, :],
                        start=(kk == 0), stop=(kk == KI2 - 1),
                        perf_mode=DR)
                t = actp.tile([128, SC], bf16, tag=f"ke{m}")
                nc.scalar.activation(t[:], psm[:], AF.Exp, scale=1.0 / WSCALE)
                return t

            def vproj(vc, m):
                psm = ps.tile([128, SC], f32, tag="ps")
                for kk in range(KI):
                    nc.tensor.matmul(
                        psm[:],
                        wv_t[:, kk, m * 128:(m + 1) * 128],
                        vc[:, kk, :],
                        start=(kk == 0), stop=(kk == KI - 1))
                return psm

            def qproj(qc, m):
                psm = ps.tile([128, SC], f32, tag="ps")
                for kk in range(KI2):
                    nc.tensor.matmul(
                        psm[:],
                        wq_t[:, 2 * kk:2 * kk + 2, m * 128:(m + 1) * 128],
                        qc[:, 2 * kk:2 * kk + 2, :],
                        start=(kk == 0), stop=(kk == KI2 - 1),
                        perf_mode=DR)
                t = sgp.tile([128, SC], bf16, tag=f"sg{m}")
                nc.scalar.activation(t[:], psm[:], AF.Sigmoid,
                                     scale=1.0 / WSCALE)
                return t

            def sk_scan(s, ke, m):
                skt = scn.tile([128, SC], f32, tag=f"sk{m}")
                init = 0.0 if s == 0 else sk_prev[m][:, SC - 1:SC]
                nc.vector.tensor_tensor_scan(
                    skt[:], ke[:], ke[:], init, OP.add, OP.bypass)
                sk_prev[m] = skt

            def kv_mul(ke, psm, m):
                kv = actp.tile([128, SC], bf16, tag=f"kv{m}")
                nc.vector.tensor_mul(kv[:], ke[:], psm[:])
                return kv

            def skv_scan(s, kv, m):
                skvt = scn.tile([128, SC], bf16, tag=f"sv{m}")
                init = 0.0 if s == 0 else skv_prev[m][:, SC - 1:SC]
                nc.vector.tensor_tensor_scan(
                    skvt[:], kv[:], kv[:], init, OP.add, OP.bypass)
                skv_prev[m] = skvt

            def ydve(state, ms, ys):
                """y = sg * skv / sk for the given m tiles (DVE)."""
                sg, sks, skvs = state
                for m in ms:
                    rcp = tmp.tile([128, SC], f32, tag="rcp")
                    nc.vector.reciprocal_approx_fast(rcp[:], sks[m][:])
                    rat = tmp.tile([128, SC], bf16, tag="rat")
                    nc.vector.tensor_mul(rat[:], skvs[m][:], rcp[:])
                    y = yp.tile([128, SC], bf16, tag=f"y{m}")
                    nc.vector.tensor_mul(y[:], rat[:], sg[m][:])
                    ys[m] = y

            def emit_out(s, ys):
                """Out-projection matmuls + psum->sbuf->dram copies for
                chunk s, interleaved so a pso buffer (bufs=2) is never
                re-tagged before its copy is in the stream.  Called with
                a 2-chunk skew and placed right after the NEXT-next
                chunk's k projection: every input (y, psum) is at least
                half a chunk old, so neither the PE nor the ACT FIFO
                ever blocks on it — in particular the ACT copies can
                never delay the next chunk's exp, whose latency gates
                the scans and the proj-psum recycling."""
                psms = []

                def one_mm(t4):
                    psm = pso.tile([128, H], f32)
                    tsl = bass.ts(t4, 128)
                    for m in range(MO):
                        for n in range(2):
                            nc.tensor.matmul(
                                psm[:, n * 512:(n + 1) * 512],
                                ys[m][:, tsl],
                                wp_t[:, m, n * 512:(n + 1) * 512],
                                start=(m == 0), stop=(m == MO - 1))
                    psms.append(psm)

                def one_cp(t4):
                    ob = osbp.tile([128, H], f32, tag="ob")
                    nc.scalar.copy(ob[:], psms[t4][:])
                    nc.scalar.dma_start(
                        out_e[s * SC + t4 * 128:s * SC + (t4 + 1) * 128, :],
                        ob[:])

                one_mm(0)
                one_mm(1)
                one_cp(0)
                one_mm(2)
                one_cp(1)
                one_mm(3)
                one_cp(2)
                one_cp(3)

            def emit_chunk(s, prev_state, out_sy=None, pre=None):
                """Emit one chunk's projections + scans, interleaved with
                the PREVIOUS chunk's y computation and out projection.

                DVE FIFO order is tuned against a ~90%-utilized DVE:
                kv muls must land early (they gate proj-PSUM recycling
                for the q projection), y(s-1) must be complete before
                the PE reaches the out-projection matmuls, and the
                scans — whose consumers are a chunk away — fill the
                remaining slack at the back.
                """
                if pre is None:
                    kc = load_in(kT_e, "ik", f8, s)
                    vc = load_in(vT_e, "iv", bf16, s)
                    qc = load_in(qT_e, "iq", f8, s)
                else:
                    kc, vc, qc = pre
                ys = [None] * MO
                ke = [kproj(kc, m) for m in range(MO)]
                if out_sy is not None:
                    emit_out(*out_sy)
                # v projections emitted pairwise with their kv muls so a
                # ps-pool (bufs=3) buffer is never re-tagged before its
                # previous consumer exists in the stream.
                psv0 = vproj(vc, 0)
                psv1 = vproj(vc, 1)
                if prev_state is not None:
                    ydve(prev_state, (0, 1), ys)
                sk_scan(s, ke[0][:], 0)
                sk_scan(s, ke[1][:], 1)
                kv0 = kv_mul(ke[0][:], psv0[:], 0)
                skv_scan(s, kv0[:], 0)
                kv1 = kv_mul(ke[1][:], psv1[:], 1)
                skv_scan(s, kv1[:], 1)
                psv2 = vproj(vc, 2)
                kv2 = kv_mul(ke[2][:], psv2[:], 2)
                psv3 = vproj(vc, 3)
                kv3 = kv_mul(ke[3][:], psv3[:], 3)
                if prev_state is not None:
                    ydve(prev_state, (2, 3), ys)
                sk_scan(s, ke[2][:], 2)
                sk_scan(s, ke[3][:], 3)
                skv_scan(s, kv2[:], 2)
                skv_scan(s, kv3[:], 3)
                sg = [qproj(qc, m) for m in range(MO)]
                state = (sg, [sk_prev[m] for m in range(MO)],
                         [skv_prev[m] for m in range(MO)])
                return state, ys

            # software pipeline: chunk s emits [kproj(s) | out(s-2) |
            # vproj+scans+y(s-1) | qproj(s)] — the out projection rides
            # two chunks behind the projections so everything it touches
            # is comfortably ready.
            state, _ = emit_chunk(0, None, pre=(kc0, vc0, qc0))
            state, yprev = emit_chunk(1, state)       # computes y(0)
            ybuf = {0: yprev}
            for s in range(2, NCH):
                state, ynew = emit_chunk(s, state,
                                         out_sy=(s - 2, ybuf.pop(s - 2)))
                ybuf[s - 1] = ynew
            emit_out(NCH - 2, ybuf.pop(NCH - 2))
            ys = [None] * MO
            ydve(state, (0, 1, 2, 3), ys)
            emit_out(NCH - 1, ys)

    nc.compile()
    prune_pe_incs(nc, verbose=True)
    return nc


def _in_maps(q, k, v, w_q, w_k, w_v, w_p):
    bf = ml_dtypes.bfloat16
    e4 = ml_dtypes.float8_e4m3
    per_b = []
    for b in range(B):
        per_b.append({
            "qT8": np.ascontiguousarray(q[b].T).astype(e4),
            "kT8": np.ascontiguousarray(k[b].T).astype(e4),
            "vT": np.ascontiguousarray(v[b].T).astype(bf),
        })
    per_g = []
    for g in range(2):
        sl = slice(g * GH, (g + 1) * GH)
        per_g.append({
            "wq8": (w_q[:, sl] * WSCALE).astype(e4),
            "wk8": (w_k[:, sl] * WSCALE).astype(e4),
            "wvb": w_v[:, sl].astype(bf),
            "wpb": w_p[sl, :].astype(bf),
        })
    return [{**per_b[c // 2], **per_g[c % 2]} for c in range(NCORES)]


def run(q, k, v, w_q, w_k, w_v, w_p, trace=False, tmpdir=None):
    if "nc" not in _cache:
        _cache["nc"] = build()
    nc = _cache["nc"]
    in_maps = _in_maps(q, k, v, w_q, w_k, w_v, w_p)
    res = run_bass_kernel_spmd(nc, in_maps, core_ids=list(range(NCORES)),
                               trace=trace, tmpdir=tmpdir)
    out = np.empty((B, S, H), np.float32)
    for b in range(B):
        out[b] = res.results[2 * b]["out"]
        out[b] += res.results[2 * b + 1]["out"]
    return out, res


def kernel(**inputs):
    out, _ = run(**{k: np.asarray(v) for k, v in inputs.items()})
    return out


# revision 21
# speedup vs baseline: 1.0330x; 1.0330x over previous
"""AFT-simple attention (nn_AsfAttention) on 8 TRN2 NeuronCores.

Reference (per batch b):
    emb_q = q @ w_q; emb_k = k @ w_k; emb_v = v @ w_v
    k_exp = exp(emb_k)
    y = sigmoid(emb_q) * cumsum(k_exp * emb_v, seq) / cumsum(k_exp, seq)
    out = y @ w_p

Sharding (v2): core c = 2*b + g handles batch b and OUT-CHANNEL half g
(512 of 1024 channels) over the FULL 8192 sequence.  The per-channel
cumsum is then fully core-local: no carry exchange, no collective, no
mid-kernel stall.  The price is that the final projection out = y @ w_p
only has half the contraction rows per core, so each core emits a
partial [S, H] output and the host sums the two partials per batch
(cheap numpy add during unshard).

Precision: q and k projections run fp8-e4m3 with DoubleRow perf mode
(2 contraction rows per PE cell -> ~1.8x matmul throughput); their
error is gated by sigmoid (q) and damped by the cumsum ratio (k),
measured 1.3e-2 rel total vs the 2e-2 budget.  v and p projections
stay bf16 (fp8 there costs ~4e-2).  Weights are pre-scaled by 128 for
fp8 (w ~ 0.01 would be subnormal in e4m3); the 1/128 unscale rides the
exp/sigmoid activation's scale input for free.

Layout: channel-major on chip ([ch partitions, seq free]) so the
projections use weights as the stationary operand, the seq cumsum is a
native tensor_tensor_scan along the free dim, and the output
projection takes y tiles as lhsT yielding natural [seq, ch_out] rows.
Everything streams in one phase: per 512-seq chunk do k/v/q
projections + scans + elementwise + out projection, with the out
projection software-pipelined one chunk behind the projections.
"""

import numpy as np
import ml_dtypes

import concourse.bass as bass
import concourse.tile as tile
from concourse import bacc, mybir
from concourse.bass_utils import run_bass_kernel_spmd

B, S, H = 4, 8192, 1024
NCORES = 8
GH = 512                  # out-channel half per core
SC = 512                  # seq chunk (columns per matmul / scan step)
NCH = S // SC             # 16 seq chunks
MO = GH // 128            # 4 out-channel tiles per core
KI = H // 128             # 8 contraction subtiles (bf16 path)
KI2 = H // 256            # 4 fp8 DoubleRow contraction pairs
NT = SC // 128            # 4 seq subtiles per chunk for the out projection
WSCALE = 128.0            # fp8 weight pre-scale

bf16 = mybir.dt.bfloat16
f8 = mybir.dt.float8e4
f32 = mybir.dt.float32
AF = mybir.ActivationFunctionType
OP = mybir.AluOpType
DR = mybir.MatmulPerfMode.DoubleRow

_cache = {}


def prune_pe_incs(nc, verbose=False):
    """Drop the per-matmul PE semaphore increment from every matmul that
    is not the end of its accumulation group (stop_tensor_calc), then
    renumber all waits on that semaphore. A wait whose original target
    inc was dropped is rounded UP to the next kept inc — safe here
    because kept-inc matmuls never depend on the rounded-up waiters
    (they only consume earlier-generation tiles).

    Each inc is a serialized EVT_SEM register write (~26 ns) on the PE
    sequencer; at ~1500 matmuls this is ~40 us of PE issue overhead.
    """
    import bisect
    from collections import defaultdict

    insts = []
    for bb in nc.main_func.blocks:
        insts.extend(bb.instructions)

    upd = defaultdict(list)
    for pos, ins in enumerate(insts):
        si = ins.sync_info
        if not si:
            continue
        for u in si.on_update:
            upd[u.ant_name].append((pos, ins, u))

    changed = 0
    for sem, us in upd.items():
        if len(us) < 64:
            continue
        if not all(type(i).__name__ == "InstMatmult"
                   and u.update_mode == "sem-inc" and u.update_value == 1
                   for _, i, u in us):
            continue
        kept = [bool(i.stop_tensor_calc) for _, i, _ in us]
        kept[-1] = True
        kept_idx = [j for j, k in enumerate(kept) if k]

        def new_thresh(t):
            j = bisect.bisect_left(kept_idx, t - 1)
            assert j < len(kept_idx), f"wait {t} beyond last kept inc"
            return j + 1

        for ins in insts:
            si = ins.sync_info
            if not si:
                continue
            for w in si.on_wait:
                if w.ant_name == sem:
                    assert w.wait_mode == "sem-ge-imm", w.wait_mode
                    w.wait_value = new_thresh(w.wait_value)
        for j, (_, ins, u) in enumerate(us):
            if not kept[j]:
                si = ins.sync_info
                rest = [x for x in si.on_update if x.ant_name != sem]
                assert len(rest) == len(si.on_update) - 1
                if rest:
                    si.on_update[:] = rest
                else:
                    si.on_update.clear()
                changed += 1
    if verbose:
        print(f"prune_pe_incs: removed {changed} matmul sem-incs")
    return changed


def build(debug=False):
    nc = bacc.Bacc("TRN2", target_bir_lowering=False, debug=debug,
                   num_devices=1)

    qT_e = nc.dram_tensor("qT8", [H, S], f8, kind="ExternalInput")
    kT_e = nc.dram_tensor("kT8", [H, S], f8, kind="ExternalInput")
    vT_e = nc.dram_tensor("vT", [H, S], bf16, kind="ExternalInput")
    wq_e = nc.dram_tensor("wq8", [H, GH], f8, kind="ExternalInput")
    wk_e = nc.dram_tensor("wk8", [H, GH], f8, kind="ExternalInput")
    wv_e = nc.dram_tensor("wvb", [H, GH], bf16, kind="ExternalInput")
    wp_e = nc.dram_tensor("wpb", [GH, H], bf16, kind="ExternalInput")
    out_e = nc.dram_tensor("out", [S, H], bf16, kind="ExternalOutput")

    with tile.TileContext(nc) as tc:
        with (
            tc.tile_pool(name="wts", bufs=1) as wts,
            tc.tile_pool(name="inb", bufs=3) as inb,
            tc.tile_pool(name="act", bufs=2) as actp,
            tc.tile_pool(name="scn", bufs=2) as scn,
            tc.tile_pool(name="sgp", bufs=2) as sgp,
            tc.tile_pool(name="yp", bufs=2) as yp,
            tc.tile_pool(name="tmp", bufs=3) as tmp,
            tc.tile_pool(name="osb", bufs=6) as osbp,
            tc.tile_pool(name="ps", bufs=4, space="PSUM") as ps,
            tc.tile_pool(name="pso", bufs=2, space="PSUM") as pso,
        ):
            # ---- weights (resident all kernel) -----------------------
            # Load order = first-use order: wk + first k chunk head the
            # DMA queues so the PE can start within a few us.
            def load_w(ext, name, dtype):
                t = wts.tile([128, KI if ext is not wp_e else MO,
                              GH if ext is not wp_e else H], dtype, tag=name)
                nsub = KI if ext is not wp_e else MO
                for kk in range(nsub):
                    nc.sync.dma_start(
                        t[:, kk, :], ext[kk * 128:(kk + 1) * 128, :])
                return t

            wk_t = load_w(wk_e, "wk", f8)

            def load_in(ext, tagpfx, dtype, s):
                t = inb.tile([128, KI, SC], dtype, tag=tagpfx)
                for kk in range(KI):
                    nc.sync.dma_start(
                        t[:, kk, :],
                        ext[kk * 128:(kk + 1) * 128, bass.ts(s, SC)])
                return t

            kc0 = load_in(kT_e, "ik", f8, 0)
            wv_t = load_w(wv_e, "wv", bf16)
            vc0 = load_in(vT_e, "iv", bf16, 0)
            wq_t = load_w(wq_e, "wq", f8)
            qc0 = load_in(qT_e, "iq", f8, 0)
            wp_t = load_w(wp_e, "wp", bf16)

            sk_prev = [None] * MO
            skv_prev = [None] * MO

            def kproj(kc, m):
                psm = ps.tile([128, SC], f32, tag="ps")
                for kk in range(KI2):
                    nc.tensor.matmul(
                        psm[:],
                        wk_t[:, 2 * kk:2 * kk + 2, m * 128:(m + 1) * 128],
                        kc[:, 2 * kk:2 * kk + 2, :],
                        start=(kk == 0), stop=(kk == KI2 - 1),
                        perf_mode=DR)
                t = actp.tile([128, SC], bf16, tag=f"ke{m}")
                nc.scalar.activation(t[:], psm[:], AF.Exp, scale=1.0 / WSCALE)
                return t

            def vproj(vc, m):
                psm = ps.tile([128, SC], f32, tag="ps")
                for kk in range(KI):
                    nc.tensor.matmul(
                        psm[:],
                        wv_t[:, kk, m * 128:(m + 1) * 128],
                        vc[:, kk, :],
                        start=(kk == 0), stop=(kk == KI - 1))
                return psm

            def qproj(qc, m):
                psm = ps.tile([128, SC], f32, tag="ps")
                for kk in range(KI2):
                    nc.tensor.matmul(
                        psm[:],
                        wq_t[:, 2 * kk:2 * kk + 2, m * 128:(m + 1) * 128],
                        qc[:, 2 * kk:2 * kk + 2, :],
                        start=(kk == 0), stop=(kk == KI2 - 1),
                        perf_mode=DR)
                t = sgp.tile([128, SC], bf16, tag=f"sg{m}")
                nc.scalar.activation(t[:], psm[:], AF.Sigmoid,
                                     scale=1.0 / WSCALE)
                return t

            def sk_scan(s, ke, m):
                skt = scn.tile([128, SC], f32, tag=f"sk{m}")
                init = 0.0 if s == 0 else sk_prev[m][:, SC - 1:SC]
                nc.vector.tensor_tensor_scan(
                    skt[:], ke[:], ke[:], init, OP.add, OP.bypass)
                sk_prev[m] = skt

            def kv_mul(ke, psm, m):
                kv = actp.tile([128, SC], bf16, tag=f"kv{m}")
                nc.vector.tensor_mul(kv[:], ke[:], psm[:])
                return kv

            def skv_scan(s, kv, m):
                skvt = scn.tile([128, SC], bf16, tag=f"sv{m}")
                init = 0.0 if s == 0 else skv_prev[m][:, SC - 1:SC]
                nc.vector.tensor_tensor_scan(
                    skvt[:], kv[:], kv[:], init, OP.add, OP.bypass)
                skv_prev[m] = skvt

            def ydve(state, ms, ys):
                """y = sg * skv / sk for the given m tiles (DVE)."""
                sg, sks, skvs = state
                for m in ms:
                    rcp = tmp.tile([128, SC], f32, tag="rcp")
                    nc.vector.reciprocal_approx_fast(rcp[:], sks[m][:])
                    rat = tmp.tile([128, SC], bf16, tag="rat")
                    nc.vector.tensor_mul(rat[:], skvs[m][:], rcp[:])
                    y = yp.tile([128, SC], bf16, tag=f"y{m}")
                    nc.vector.tensor_mul(y[:], rat[:], sg[m][:])
                    ys[m] = y

            def emit_out(s, ys):
                """Out-projection matmuls + psum->sbuf->dram copies for
                chunk s, interleaved so a pso buffer (bufs=2) is never
                re-tagged before its copy is in the stream.  Called with
                a 2-chunk skew and placed right after the NEXT-next
                chunk's k projection: every input (y, psum) is at least
                half a chunk old, so neither the PE nor the ACT FIFO
                ever blocks on it — in particular the ACT copies can
                never delay the next chunk's exp, whose latency gates
                the scans and the proj-psum recycling."""
                psms = []

                def one_mm(t4):
                    psm = pso.tile([128, H], f32)
                    tsl = bass.ts(t4, 128)
                    for m in range(MO):
                        for n in range(2):
                            nc.tensor.matmul(
                                psm[:, n * 512:(n + 1) * 512],
                                ys[m][:, tsl],
                                wp_t[:, m, n * 512:(n + 1) * 512],
                                start=(m == 0), stop=(m == MO - 1))
                    psms.append(psm)

                def one_cp(t4):
                    ob = osbp.tile([128, H], bf16, tag="ob")
                    nc.scalar.copy(ob[:], psms[t4][:])
                    nc.scalar.dma_start(
                        out_e[s * SC + t4 * 128:s * SC + (t4 + 1) * 128, :],
                        ob[:])

                one_mm(0)
                one_mm(1)
                one_cp(0)
                one_mm(2)
                one_cp(1)
                one_mm(3)
                one_cp(2)
                one_cp(3)

            def emit_chunk(s, prev_state, out_sy=None, pre=None):
                """Emit one chunk's projections + scans, interleaved with
                the PREVIOUS chunk's y computation and out projection.

                DVE FIFO order is tuned against a ~90%-utilized DVE:
                kv muls must land early (they gate proj-PSUM recycling
                for the q projection), y(s-1) must be complete before
                the PE reaches the out-projection matmuls, and the
                scans — whose consumers are a chunk away — fill the
                remaining slack at the back.
                """
                if pre is None:
                    kc = load_in(kT_e, "ik", f8, s)
                    vc = load_in(vT_e, "iv", bf16, s)
                    qc = load_in(qT_e, "iq", f8, s)
                else:
                    kc, vc, qc = pre
                ys = [None] * MO
                ke = [kproj(kc, m) for m in range(MO)]
                if out_sy is not None:
                    emit_out(*out_sy)
                # v projections emitted pairwise with their kv muls so a
                # ps-pool (bufs=3) buffer is never re-tagged before its
                # previous consumer exists in the stream.
                psv0 = vproj(vc, 0)
                psv1 = vproj(vc, 1)
                if prev_state is not None:
                    ydve(prev_state, (0, 1), ys)
                sk_scan(s, ke[0][:], 0)
                sk_scan(s, ke[1][:], 1)
                kv0 = kv_mul(ke[0][:], psv0[:], 0)
                skv_scan(s, kv0[:], 0)
                kv1 = kv_mul(ke[1][:], psv1[:], 1)
                skv_scan(s, kv1[:], 1)
                psv2 = vproj(vc, 2)
                kv2 = kv_mul(ke[2][:], psv2[:], 2)
                psv3 = vproj(vc, 3)
                kv3 = kv_mul(ke[3][:], psv3[:], 3)
                if prev_state is not None:
                    ydve(prev_state, (2, 3), ys)
                sk_scan(s, ke[2][:], 2)
                sk_scan(s, ke[3][:], 3)
                skv_scan(s, kv2[:], 2)
                skv_scan(s, kv3[:], 3)
                sg = [qproj(qc, m) for m in range(MO)]
                state = (sg, [sk_prev[m] for m in range(MO)],
                         [skv_prev[m] for m in range(MO)])
                return state, ys

            # software pipeline: chunk s emits [kproj(s) | out(s-2) |
            # vproj+scans+y(s-1) | qproj(s)] — the out projection rides
            # two chunks behind the projections so everything it touches
            # is comfortably ready.
            state, _ = emit_chunk(0, None, pre=(kc0, vc0, qc0))
            state, yprev = emit_chunk(1, state)       # computes y(0)
            ybuf = {0: yprev}
            for s in range(2, NCH):
                state, ynew = emit_chunk(s, state,
                                         out_sy=(s - 2, ybuf.pop(s - 2)))
                ybuf[s - 1] = ynew
            # tail: out(14) immediately after chunk 15's emission (its
            # y was just computed), then y(15) + out(15)
            emit_out(NCH - 2, ybuf.pop(NCH - 2))
            ys = [None] * MO
            ydve(state, (0, 1, 2, 3), ys)
            emit_out(NCH - 1, ys)

    nc.compile()
    prune_pe_incs(nc, verbose=True)
    return nc


def _in_maps(q, k, v, w_q, w_k, w_v, w_p):
    bf = ml_dtypes.bfloat16
    e4 = ml_dtypes.float8_e4m3
    per_b = []
    for b in range(B):
        per_b.append({
            "qT8": np.ascontiguousarray(q[b].T).astype(e4),
            "kT8": np.ascontiguousarray(k[b].T).astype(e4),
            "vT": np.ascontiguousarray(v[b].T).astype(bf),
        })
    per_g = []
    for g in range(2):
        sl = slice(g * GH, (g + 1) * GH)
        per_g.append({
            "wq8": (w_q[:, sl] * WSCALE).astype(e4),
            "wk8": (w_k[:, sl] * WSCALE).astype(e4),
            "wvb": w_v[:, sl].astype(bf),
            "wpb": w_p[sl, :].astype(bf),
        })
    return [{**per_b[c // 2], **per_g[c % 2]} for c in range(NCORES)]


def run(q, k, v, w_q, w_k, w_v, w_p, trace=False, tmpdir=None):
    if "nc" not in _cache:
        _cache["nc"] = build()
    nc = _cache["nc"]
    in_maps = _in_maps(q, k, v, w_q, w_k, w_v, w_p)
    res = run_bass_kernel_spmd(nc, in_maps, core_ids=list(range(NCORES)),
                               trace=trace, tmpdir=tmpdir)
    out = np.empty((B, S, H), np.float32)
    for b in range(B):
        out[b] = res.results[2 * b]["out"].astype(np.float32)
        out[b] += res.results[2 * b + 1]["out"].astype(np.float32)
    return out, res


def kernel(**inputs):
    out, _ = run(**{k: np.asarray(v) for k, v in inputs.items()})
    return out


# revision 23
# speedup vs baseline: 1.1489x; 1.1122x over previous
"""AFT-simple attention (nn_AsfAttention) on 8 TRN2 NeuronCores.

Reference (per batch b):
    emb_q = q @ w_q; emb_k = k @ w_k; emb_v = v @ w_v
    k_exp = exp(emb_k)
    y = sigmoid(emb_q) * cumsum(k_exp * emb_v, seq) / cumsum(k_exp, seq)
    out = y @ w_p

Sharding (v2): core c = 2*b + g handles batch b and OUT-CHANNEL half g
(512 of 1024 channels) over the FULL 8192 sequence.  The per-channel
cumsum is then fully core-local: no carry exchange, no collective, no
mid-kernel stall.  The price is that the final projection out = y @ w_p
only has half the contraction rows per core, so each core emits a
partial [S, H] output and the host sums the two partials per batch
(cheap numpy add during unshard).

Precision: q and k projections run fp8-e4m3 with DoubleRow perf mode
(2 contraction rows per PE cell -> ~1.8x matmul throughput); their
error is gated by sigmoid (q) and damped by the cumsum ratio (k),
measured 1.3e-2 rel total vs the 2e-2 budget.  v and p projections
stay bf16 (fp8 there costs ~4e-2).  Weights are pre-scaled by 128 for
fp8 (w ~ 0.01 would be subnormal in e4m3); the 1/128 unscale rides the
exp/sigmoid activation's scale input for free.

Layout: channel-major on chip ([ch partitions, seq free]) so the
projections use weights as the stationary operand, the seq cumsum is a
native tensor_tensor_scan along the free dim, and the output
projection takes y tiles as lhsT yielding natural [seq, ch_out] rows.
Everything streams in one phase: per 512-seq chunk do k/v/q
projections + scans + elementwise + out projection, with the out
projection software-pipelined one chunk behind the projections.
"""

import numpy as np
import ml_dtypes

import concourse.bass as bass
import concourse.tile as tile
from concourse import bacc, mybir
from concourse.bass_utils import run_bass_kernel_spmd

B, S, H = 4, 8192, 1024
NCORES = 8
GH = 512                  # out-channel half per core
SC = 512                  # seq chunk (columns per matmul / scan step)
NCH = S // SC             # 16 seq chunks
MO = GH // 128            # 4 out-channel tiles per core
KI = H // 128             # 8 contraction subtiles (bf16 path)
KI2 = H // 256            # 4 fp8 DoubleRow contraction pairs
NT = SC // 128            # 4 seq subtiles per chunk for the out projection
WSCALE = 128.0            # fp8 weight pre-scale

bf16 = mybir.dt.bfloat16
f8 = mybir.dt.float8e4
f32 = mybir.dt.float32
AF = mybir.ActivationFunctionType
OP = mybir.AluOpType
DR = mybir.MatmulPerfMode.DoubleRow

_cache = {}


def prune_pe_incs(nc, verbose=False):
    """Drop the per-matmul PE semaphore increment from every matmul that
    is not the end of its accumulation group (stop_tensor_calc), then
    renumber all waits on that semaphore. A wait whose original target
    inc was dropped is rounded UP to the next kept inc — safe here
    because kept-inc matmuls never depend on the rounded-up waiters
    (they only consume earlier-generation tiles).

    Each inc is a serialized EVT_SEM register write (~26 ns) on the PE
    sequencer; at ~1500 matmuls this is ~40 us of PE issue overhead.
    """
    import bisect
    from collections import defaultdict

    insts = []
    for bb in nc.main_func.blocks:
        insts.extend(bb.instructions)

    upd = defaultdict(list)
    for pos, ins in enumerate(insts):
        si = ins.sync_info
        if not si:
            continue
        for u in si.on_update:
            upd[u.ant_name].append((pos, ins, u))

    changed = 0
    for sem, us in upd.items():
        if len(us) < 64:
            continue
        if not all(type(i).__name__ == "InstMatmult"
                   and u.update_mode == "sem-inc" and u.update_value == 1
                   for _, i, u in us):
            continue
        kept = [bool(i.stop_tensor_calc) for _, i, _ in us]
        kept[-1] = True
        kept_idx = [j for j, k in enumerate(kept) if k]

        def new_thresh(t):
            j = bisect.bisect_left(kept_idx, t - 1)
            assert j < len(kept_idx), f"wait {t} beyond last kept inc"
            return j + 1

        for ins in insts:
            si = ins.sync_info
            if not si:
                continue
            for w in si.on_wait:
                if w.ant_name == sem:
                    assert w.wait_mode == "sem-ge-imm", w.wait_mode
                    w.wait_value = new_thresh(w.wait_value)
        for j, (_, ins, u) in enumerate(us):
            if not kept[j]:
                si = ins.sync_info
                rest = [x for x in si.on_update if x.ant_name != sem]
                assert len(rest) == len(si.on_update) - 1
                if rest:
                    si.on_update[:] = rest
                else:
                    si.on_update.clear()
                changed += 1
    if verbose:
        print(f"prune_pe_incs: removed {changed} matmul sem-incs")
    return changed


def build(debug=False):
    nc = bacc.Bacc("TRN2", target_bir_lowering=False, debug=debug,
                   num_devices=1)

    qT_e = nc.dram_tensor("qT8", [H, S], f8, kind="ExternalInput")
    kT_e = nc.dram_tensor("kT8", [H, S], f8, kind="ExternalInput")
    vT_e = nc.dram_tensor("vT", [H, S], bf16, kind="ExternalInput")
    wq_e = nc.dram_tensor("wq8", [H, GH], f8, kind="ExternalInput")
    wk_e = nc.dram_tensor("wk8", [H, GH], f8, kind="ExternalInput")
    wv_e = nc.dram_tensor("wvb", [H, GH], bf16, kind="ExternalInput")
    wp_e = nc.dram_tensor("wpb", [GH, H], bf16, kind="ExternalInput")
    out_e = nc.dram_tensor("out", [S, H], bf16, kind="ExternalOutput")

    with tile.TileContext(nc) as tc:
        with (
            tc.tile_pool(name="wts", bufs=1) as wts,
            tc.tile_pool(name="inb", bufs=3) as inb,
            tc.tile_pool(name="act", bufs=2) as actp,
            tc.tile_pool(name="scn", bufs=2) as scn,
            tc.tile_pool(name="sgp", bufs=2) as sgp,
            tc.tile_pool(name="yp", bufs=2) as yp,
            tc.tile_pool(name="tmp", bufs=3) as tmp,
            tc.tile_pool(name="osb", bufs=6) as osbp,
            tc.tile_pool(name="ps", bufs=4, space="PSUM") as ps,
            tc.tile_pool(name="pso", bufs=2, space="PSUM") as pso,
        ):
            # ---- weights (resident all kernel) -----------------------
            # Load order = first-use order: wk + first k chunk head the
            # DMA queues so the PE can start within a few us.
            def load_w(ext, name, dtype):
                t = wts.tile([128, KI if ext is not wp_e else MO,
                              GH if ext is not wp_e else H], dtype, tag=name)
                nsub = KI if ext is not wp_e else MO
                for kk in range(nsub):
                    nc.sync.dma_start(
                        t[:, kk, :], ext[kk * 128:(kk + 1) * 128, :])
                return t

            wk_t = load_w(wk_e, "wk", f8)

            def load_in(ext, tagpfx, dtype, off, w):
                t = inb.tile([128, KI, SC], dtype, tag=tagpfx)
                for kk in range(KI):
                    nc.sync.dma_start(
                        t[:, kk, :w],
                        ext[kk * 128:(kk + 1) * 128, off:off + w])
                return t

            kc0 = load_in(kT_e, "ik", f8, 0, 256)
            wv_t = load_w(wv_e, "wv", bf16)
            vc0 = load_in(vT_e, "iv", bf16, 0, 256)
            wq_t = load_w(wq_e, "wq", f8)
            qc0 = load_in(qT_e, "iq", f8, 0, 256)
            wp_t = load_w(wp_e, "wp", bf16)

            sk_prev = [None] * MO
            skv_prev = [None] * MO
            w_prev = [SC] * MO

            def kproj(kc, m, w):
                psm = ps.tile([128, SC], f32, tag="ps")
                for kk in range(KI2):
                    nc.tensor.matmul(
                        psm[:, :w],
                        wk_t[:, 2 * kk:2 * kk + 2, m * 128:(m + 1) * 128],
                        kc[:, 2 * kk:2 * kk + 2, :w],
                        start=(kk == 0), stop=(kk == KI2 - 1),
                        perf_mode=DR)
                t = actp.tile([128, SC], bf16, tag=f"ke{m}")
                nc.scalar.activation(t[:, :w], psm[:, :w], AF.Exp,
                                     scale=1.0 / WSCALE)
                return t

            def vproj(vc, m, w):
                psm = ps.tile([128, SC], f32, tag="ps")
                for kk in range(KI):
                    nc.tensor.matmul(
                        psm[:, :w],
                        wv_t[:, kk, m * 128:(m + 1) * 128],
                        vc[:, kk, :w],
                        start=(kk == 0), stop=(kk == KI - 1))
                return psm

            def qproj(qc, m, w):
                psm = ps.tile([128, SC], f32, tag="ps")
                for kk in range(KI2):
                    nc.tensor.matmul(
                        psm[:, :w],
                        wq_t[:, 2 * kk:2 * kk + 2, m * 128:(m + 1) * 128],
                        qc[:, 2 * kk:2 * kk + 2, :w],
                        start=(kk == 0), stop=(kk == KI2 - 1),
                        perf_mode=DR)
                t = sgp.tile([128, SC], bf16, tag=f"sg{m}")
                nc.scalar.activation(t[:, :w], psm[:, :w], AF.Sigmoid,
                                     scale=1.0 / WSCALE)
                return t

            def sk_scan(s, ke, m, w):
                skt = scn.tile([128, SC], f32, tag=f"sk{m}")
                init = (0.0 if s == 0
                        else sk_prev[m][:, w_prev[m] - 1:w_prev[m]])
                nc.vector.tensor_tensor_scan(
                    skt[:, :w], ke[:, :w], ke[:, :w], init,
                    OP.add, OP.bypass)
                sk_prev[m] = skt

            def kv_mul(ke, psm, m, w):
                kv = actp.tile([128, SC], bf16, tag=f"kv{m}")
                nc.vector.tensor_mul(kv[:, :w], ke[:, :w], psm[:, :w])
                return kv

            def skv_scan(s, kv, m, w):
                skvt = scn.tile([128, SC], bf16, tag=f"sv{m}")
                init = (0.0 if s == 0
                        else skv_prev[m][:, w_prev[m] - 1:w_prev[m]])
                nc.vector.tensor_tensor_scan(
                    skvt[:, :w], kv[:, :w], kv[:, :w], init,
                    OP.add, OP.bypass)
                skv_prev[m] = skvt
                w_prev[m] = w

            def ydve(state, ms, ys):
                """y = sg * skv / sk for the given m tiles (DVE)."""
                sg, sks, skvs, w = state
                for m in ms:
                    rcp = tmp.tile([128, SC], f32, tag="rcp")
                    nc.vector.reciprocal_approx_fast(rcp[:, :w],
                                                     sks[m][:, :w])
                    rat = tmp.tile([128, SC], bf16, tag="rat")
                    nc.vector.tensor_mul(rat[:, :w], skvs[m][:, :w],
                                         rcp[:, :w])
                    y = yp.tile([128, SC], bf16, tag=f"y{m}")
                    nc.vector.tensor_mul(y[:, :w], rat[:, :w],
                                         sg[m][:, :w])
                    ys[m] = y

            def emit_out(off, w, ys):
                """Out-projection matmuls + psum->sbuf->dram copies for
                the chunk at seq offset `off`, interleaved so a pso
                buffer (bufs=2) is never re-tagged before its copy is in
                the stream.  Called with a 2-chunk skew and placed right
                after the NEXT-next chunk's k projection: every input
                (y, psum) is at least half a chunk old, so neither the
                PE nor the ACT FIFO ever blocks on it — in particular
                the ACT copies can never delay the next chunk's exp,
                whose latency gates the scans and proj-psum recycling."""
                nt = w // 128
                psms = []

                def one_mm(t4):
                    psm = pso.tile([128, H], f32)
                    tsl = bass.ts(t4, 128)
                    for m in range(MO):
                        for n in range(2):
                            nc.tensor.matmul(
                                psm[:, n * 512:(n + 1) * 512],
                                ys[m][:, tsl],
                                wp_t[:, m, n * 512:(n + 1) * 512],
                                start=(m == 0), stop=(m == MO - 1))
                    psms.append(psm)

                def one_cp(t4):
                    ob = osbp.tile([128, H], bf16, tag="ob")
                    nc.scalar.copy(ob[:], psms[t4][:])
                    nc.scalar.dma_start(
                        out_e[off + t4 * 128:off + (t4 + 1) * 128, :],
                        ob[:])

                if nt == 4:
                    one_mm(0)
                    one_mm(1)
                    one_cp(0)
                    one_mm(2)
                    one_cp(1)
                    one_mm(3)
                    one_cp(2)
                    one_cp(3)
                else:
                    for t4 in range(nt):
                        one_mm(t4)
                    for t4 in range(nt):
                        one_cp(t4)

            def emit_chunk(s, off, w, prev_state, out_owy=None, pre=None):
                """Emit one chunk's projections + scans, interleaved with
                the PREVIOUS chunk's y computation and the out projection
                from two chunks back.

                DVE FIFO order is tuned against a ~90%-utilized DVE:
                kv muls must land early (they gate proj-PSUM recycling
                for the q projection), y(s-1) must be complete before
                the PE reaches the out-projection matmuls, and the
                scans — whose consumers are a chunk away — fill the
                remaining slack at the back.
                """
                if pre is None:
                    kc = load_in(kT_e, "ik", f8, off, w)
                    vc = load_in(vT_e, "iv", bf16, off, w)
                    qc = load_in(qT_e, "iq", f8, off, w)
                else:
                    kc, vc, qc = pre
                ys = [None] * MO
                ke = [kproj(kc, m, w) for m in range(MO)]
                if out_owy is not None:
                    emit_out(*out_owy)
                # v projections emitted pairwise with their kv muls so a
                # ps-pool buffer is never re-tagged before its previous
                # consumer exists in the stream.
                psv0 = vproj(vc, 0, w)
                psv1 = vproj(vc, 1, w)
                if prev_state is not None:
                    ydve(prev_state, (0, 1), ys)
                sk_scan(s, ke[0][:], 0, w)
                sk_scan(s, ke[1][:], 1, w)
                kv0 = kv_mul(ke[0][:], psv0[:], 0, w)
                skv_scan(s, kv0[:], 0, w)
                kv1 = kv_mul(ke[1][:], psv1[:], 1, w)
                skv_scan(s, kv1[:], 1, w)
                psv2 = vproj(vc, 2, w)
                kv2 = kv_mul(ke[2][:], psv2[:], 2, w)
                psv3 = vproj(vc, 3, w)
                kv3 = kv_mul(ke[3][:], psv3[:], 3, w)
                if prev_state is not None:
                    ydve(prev_state, (2, 3), ys)
                sk_scan(s, ke[2][:], 2, w)
                sk_scan(s, ke[3][:], 3, w)
                skv_scan(s, kv2[:], 2, w)
                skv_scan(s, kv3[:], 3, w)
                sg = [qproj(qc, m, w) for m in range(MO)]
                state = (sg, [sk_prev[m] for m in range(MO)],
                         [skv_prev[m] for m in range(MO)], w)
                return state, ys

            # Chunk schedule: narrow chunks at the head (earlier first
            # matmul behind the DMA ramp) and at the tail (the last two
            # out projections are pure tail — halving them halves it).
            widths = [256, 256] + [SC] * 14 + [256, 256]
            offs = [sum(widths[:i]) for i in range(len(widths))]
            nch = len(widths)
            assert sum(widths) == S

            # software pipeline: chunk s emits [kproj(s) | out(s-2) |
            # vproj+scans+y(s-1) | qproj(s)] — the out projection rides
            # two chunks behind the projections so everything it touches
            # is comfortably ready.
            state, _ = emit_chunk(0, offs[0], widths[0], None,
                                  pre=(kc0, vc0, qc0))
            state, yprev = emit_chunk(1, offs[1], widths[1], state)
            ybuf = {0: yprev}
            for s in range(2, nch):
                state, ynew = emit_chunk(
                    s, offs[s], widths[s], state,
                    out_owy=(offs[s - 2], widths[s - 2], ybuf.pop(s - 2)))
                ybuf[s - 1] = ynew
            # tail: out(n-2) immediately after the last chunk's emission
            # (its y was just computed), then y(n-1) + out(n-1)
            emit_out(offs[nch - 2], widths[nch - 2], ybuf.pop(nch - 2))
            ys = [None] * MO
            ydve(state, (0, 1, 2, 3), ys)
            emit_out(offs[nch - 1], widths[nch - 1], ys)

    nc.compile()
    prune_pe_incs(nc, verbose=True)
    return nc


def _in_maps(q, k, v, w_q, w_k, w_v, w_p):
    bf = ml_dtypes.bfloat16
    e4 = ml_dtypes.float8_e4m3
    per_b = []
    for b in range(B):
        per_b.append({
            "qT8": np.ascontiguousarray(q[b].T).astype(e4),
            "kT8": np.ascontiguousarray(k[b].T).astype(e4),
            "vT": np.ascontiguousarray(v[b].T).astype(bf),
        })
    per_g = []
    for g in range(2):
        sl = slice(g * GH, (g + 1) * GH)
        per_g.append({
            "wq8": (w_q[:, sl] * WSCALE).astype(e4),
            "wk8": (w_k[:, sl] * WSCALE).astype(e4),
            "wvb": w_v[:, sl].astype(bf),
            "wpb": w_p[sl, :].astype(bf),
        })
    return [{**per_b[c // 2], **per_g[c % 2]} for c in range(NCORES)]


def run(q, k, v, w_q, w_k, w_v, w_p, trace=False, tmpdir=None):
    if "nc" not in _cache:
        _cache["nc"] = build()
    nc = _cache["nc"]
    in_maps = _in_maps(q, k, v, w_q, w_k, w_v, w_p)
    res = run_bass_kernel_spmd(nc, in_maps, core_ids=list(range(NCORES)),
                               trace=trace, tmpdir=tmpdir)
    out = np.empty((B, S, H), np.float32)
    for b in range(B):
        out[b] = res.results[2 * b]["out"].astype(np.float32)
        out[b] += res.results[2 * b + 1]["out"].astype(np.float32)
    return out, res


def kernel(**inputs):
    out, _ = run(**{k: np.asarray(v) for k, v in inputs.items()})
    return out
